# revision 11
# baseline (speedup 1.0000x reference)
"""Trainium2 Bass kernel for nn_EruSelfAttentionModel (B=4,S=1024,E=1024,A=64,H=16,L=2).

Sharding: 8 cores; core c handles batch c//2 and heads (c%2)*8..(c%2)*8+8.
Heads are independent through both layers, so each core runs its (batch,
8-head) slice end-to-end with no collectives.

v2 (this file): all matmuls in bf16 (4x PE throughput vs fp32), both layers
fused per head (no DRAM round-trip for the inter-layer activations), and the
layer-1 softmax division is folded into the inter-layer LayerNorm via LN's
scale invariance: LN(out/z) == LN(out_raw) with the eps bias corrected to
eps*z^2 per row (z columns obtained by PE-transposing the ones-matmul z).

Per-core dataflow per head:
  stage0 (once): indirect-DMA gather of bf16 embeddings -> LN (bn_stats) ->
      PE transpose -> hn0T [E,S] bf16 in SBUF (shared by all 8 heads).
  layer 1: qkT / scores+exp(wT bf16) / v(bf16) / z (fp32r ones-matmul) /
      out_raw [S,E] fp32 in SBUF -> LN (eps*z^2 bias) -> PE transpose ->
      hn1T [E,S] bf16.
  layer 2: same attention, final out = psum * (1/z) column, fp32 -> DRAM.
"""

import math
import os
from contextlib import ExitStack

import numpy as np
import ml_dtypes

# The device path (bass2jax under axon) needs the axon PJRT backend; a
# JAX_PLATFORMS=cpu pin (common for running the jax reference) would break it.
if "JAX_PLATFORMS" in os.environ and "axon" not in os.environ["JAX_PLATFORMS"]:
    del os.environ["JAX_PLATFORMS"]

import bass_rust
from bass_rust import SyncInfo
import concourse.bass as bass
import concourse.mybir as mybir
import concourse.tile as tile
from concourse.bass_utils import run_bass_kernel_spmd
from concourse.masks import make_identity

B, S, E, A, H, L, V = 4, 1024, 1024, 64, 16, 2, 32000
EPS = 1e-5
SCALE = math.sqrt(E)
P = 128
KO = E // P       # 8 k-blocks over E
SB = S // P       # 8 s-blocks
NH = H // 2       # 8 heads per core
HALF = S // 2     # 512
TA = 2 * A        # 128 (packed q|k)
FP = mybir.dt.float32
FR = mybir.dt.float32r
BF = mybir.dt.bfloat16
AF = mybir.ActivationFunctionType
OP = mybir.AluOpType

_WID = [0]


def _legalize_multi_waits(nc, max_keep=1):
    """This walrus build accepts at most one sync-wait command per engine
    instruction; split extras into standalone EventSemaphore waits."""
    for f in nc.m.functions:
        for blk in f.blocks:
            out = []
            changed = False
            for inst in blk.instructions:
                si = inst.sync_info
                ow = list(si.on_wait) if si is not None else []
                if len(ow) > max_keep:
                    changed = True
                    for w in ow[:-max_keep]:
                        _WID[0] += 1
                        out.append(bass_rust.InstEventSemaphore(
                            name=f"WSPLIT-{_WID[0]}",
                            engine=inst.engine,
                            ins=[], outs=[],
                            sync_info=SyncInfo(on_wait=[w], on_update=[]),
                        ))
                    inst.sync_info = SyncInfo(on_wait=ow[-max_keep:],
                                              on_update=list(si.on_update))
                out.append(inst)
            if changed:
                blk.instructions = out


def _build_nc(g0_identity, g1_identity, legalize=True):
    nc = bass.Bass("TRN2")

    emb = nc.dram_tensor("emb", [V, E], BF, kind="ExternalInput")
    xidx = nc.dram_tensor("xidx", [S, 1], mybir.dt.int32, kind="ExternalInput")
    wqk = nc.dram_tensor("wqk", [L, NH, E, TA], BF, kind="ExternalInput")
    wv = nc.dram_tensor("wv", [L, NH, E, E], BF, kind="ExternalInput")
    g0 = nc.dram_tensor("g0", [E], FP, kind="ExternalInput")
    b0 = nc.dram_tensor("b0", [E], FP, kind="ExternalInput")
    g1 = nc.dram_tensor("g1", [E], FP, kind="ExternalInput")
    b1 = nc.dram_tensor("b1", [E], FP, kind="ExternalInput")
    out_d = nc.dram_tensor("out", [NH, S, E], FP, kind="ExternalOutput")

    with tile.TileContext(nc) as tc, ExitStack() as ctx:
        const = ctx.enter_context(tc.tile_pool(name="const", bufs=1))
        hn0p = ctx.enter_context(tc.tile_pool(name="hn0p", bufs=1))
        hn1p = ctx.enter_context(tc.tile_pool(name="hn1p", bufs=1))
        wqkp = ctx.enter_context(tc.tile_pool(name="wqkp", bufs=2))
        wvp = ctx.enter_context(tc.tile_pool(name="wvp", bufs=2))
        vp = ctx.enter_context(tc.tile_pool(name="vp", bufs=2))
        wtp = ctx.enter_context(tc.tile_pool(name="wtp", bufs=2))
        qkp = ctx.enter_context(tc.tile_pool(name="qkp", bufs=2))
        htp = ctx.enter_context(tc.tile_pool(name="htp", bufs=1))
        hnsp = ctx.enter_context(tc.tile_pool(name="hnsp", bufs=1))
        stp = ctx.enter_context(tc.tile_pool(name="stp", bufs=2))
        otp = ctx.enter_context(tc.tile_pool(name="otp", bufs=2))
        lnp = ctx.enter_context(tc.tile_pool(name="lnp", bufs=1))
        psB = ctx.enter_context(tc.tile_pool(name="psB", bufs=3, space="PSUM"))
        psS = ctx.enter_context(tc.tile_pool(name="psS", bufs=2, space="PSUM"))
        psZ = ctx.enter_context(tc.tile_pool(name="psZ", bufs=2, space="PSUM"))
        psT = ctx.enter_context(tc.tile_pool(name="psT", bufs=1, space="PSUM"))

        identB = const.tile([P, P], BF)
        make_identity(nc, identB[:])
        identF = const.tile([P, P], FP)
        make_identity(nc, identF[:])
        onesR = const.tile([P, P], FR)
        onesF = const.tile([P, P], FP)
        nc.vector.memset(onesF[:], 1.0)
        nc.vector.tensor_copy(onesR[:], onesF[:])
        oneER = const.tile([P, P], FR)
        nc.vector.memset(onesF[:], 1.0 / E)
        nc.vector.tensor_copy(oneER[:], onesF[:])
        nc.vector.memset(onesF[:], 1.0)
        eps_t = const.tile([P, 1], FP)
        nc.vector.memset(eps_t[:], EPS)
        if not g0_identity:
            g0rep = const.tile([P, E], FP)
            b0rep = const.tile([P, E], FP)
            nc.sync.dma_start(g0rep[:], g0.ap()[None, :].to_broadcast([P, E]))
            nc.sync.dma_start(b0rep[:], b0.ap()[None, :].to_broadcast([P, E]))
        if not g1_identity:
            g1c_t = const.tile([P, KO], FP)
            b1c_t = const.tile([P, KO], FP)
            nc.sync.dma_start(g1c_t[:], g1.ap().rearrange("(ko p) -> p ko", p=P))
            nc.sync.dma_start(b1c_t[:], b1.ap().rearrange("(ko p) -> p ko", p=P))

        # ---------------- stage 0: embed + LN0 + transpose -> hn0T ----------
        hn0T = hn0p.tile([P, KO, S], BF, tag="hn0")  # [e_inner, e_outer, s]
        for sb in range(SB):
            idxt = stp.tile([P, 1], mybir.dt.int32, tag="idx")
            nc.sync.dma_start(idxt[:], xidx[sb * P:(sb + 1) * P, :])
            h0sb = hnsp.tile([P, E], BF, tag="h0")
            nc.gpsimd.indirect_dma_start(
                out=h0sb[:], out_offset=None, in_=emb[:, :],
                in_offset=bass.IndirectOffsetOnAxis(ap=idxt[:, :1], axis=0),
            )
            stats = stp.tile([P, 2, 6], FP, tag="bnst")
            nc.vector.bn_stats(stats[:, 0, :], h0sb[:, 0:HALF])
            nc.vector.bn_stats(stats[:, 1, :], h0sb[:, HALF:S])
            mv = stp.tile([P, 2], FP, tag="bnmv")
            nc.vector.bn_aggr(mv[:], stats[:])
            sd = stp.tile([P, 1], FP, tag="sd")
            nc.scalar.activation(sd[:], mv[:, 1:2], AF.Sqrt, bias=eps_t[:])
            rstd = stp.tile([P, 1], FP, tag="rstd")
            nc.vector.reciprocal(rstd[:], sd[:])
            nc.vector.tensor_scalar(h0sb[:], h0sb[:], scalar1=mv[:, 0:1],
                                    scalar2=rstd[:], op0=OP.subtract,
                                    op1=OP.mult)
            if not g0_identity:
                nc.vector.tensor_tensor(h0sb[:], h0sb[:], g0rep[:], OP.mult)
                nc.vector.tensor_tensor(h0sb[:], h0sb[:], b0rep[:], OP.add)
            for eo in range(0, KO, 4):
                pst = psT.tile([P, 4, P], BF, tag="pst")
                for j in range(4):
                    nc.tensor.transpose(pst[:, j, :],
                                        h0sb[:, (eo + j) * P:(eo + j + 1) * P],
                                        identB[:])
                nc.any.tensor_copy(hn0T[:, eo:eo + 4, sb * P:(sb + 1) * P],
                                   pst[:, :, :])

        # ---------------- attention unit ------------------------------------
        def attn_unit(layer, head, hn, final):
            """hn: [P, KO, S] bf16 ([E,S] layout). If not final, produces the
            next layer's hn ([E,S] bf16) via the fused z-free LayerNorm
            (rstd = 1/sqrt(var_raw + eps*z^2), layer-1 out kept un-normalized
            in [E,S] layout so no transposes are needed); if final, scales by
            1/z and DMAs [S,E] fp32 to out_d."""
            wqk_sb = wqkp.tile([P, KO, TA], BF, tag="wqk")
            nc.sync.dma_start(wqk_sb[:],
                              wqk.ap()[layer, head].rearrange("(ko p) m -> p ko m", p=P))
            wv_sb = wvp.tile([P, KO, E], BF, tag="wv")
            nc.sync.dma_start(wv_sb[:],
                              wv.ap()[layer, head].rearrange("(ko p) o -> p ko o", p=P))

            # qkT: q rows 0:64 of qT; k rows 0:64 of kT. qT rows 64:128 carry
            # junk (finite) and kT rows 64:128 are zeroed, so the K=128
            # scores matmul adds an exact 0 for the pad rows while keeping
            # the fast-weight-load path (needs 128 weight rows).
            qT = qkp.tile([P, S], BF, tag="qT")
            kT = qkp.tile([P, S], BF, tag="kT")
            nc.gpsimd.memset(kT[A:P, :], 0.0)
            for nb in range(2):
                ps_qk = psB.tile([P, HALF], FP, tag="big")
                for ko in range(KO):
                    nc.tensor.matmul(ps_qk[:],
                                     lhsT=wqk_sb[:, ko, :],
                                     rhs=hn[:, ko, nb * HALF:(nb + 1) * HALF],
                                     start=(ko == 0), stop=(ko == KO - 1))
                nc.scalar.copy(qT[:, nb * HALF:(nb + 1) * HALF], ps_qk[:, :])
                nc.scalar.copy(kT[0:A, nb * HALF:(nb + 1) * HALF], ps_qk[A:TA, :])

            # scoresT + exp -> wT [t_inner, tb, s] bf16
            wT = wtp.tile([P, SB, S], BF, tag="wT")
            for sh in range(2):
                for tb in range(SB):
                    ps_s = psS.tile([P, HALF], FP, tag="s")
                    nc.tensor.matmul(ps_s[:], lhsT=kT[:, tb * P:(tb + 1) * P],
                                     rhs=qT[:, sh * HALF:(sh + 1) * HALF],
                                     start=True, stop=True)
                    nc.scalar.activation(wT[:, tb, sh * HALF:(sh + 1) * HALF],
                                         ps_s[:], AF.Exp,
                                         scale=float(1.0 / SCALE))

            # v: [t_inner, tb, o] bf16
            v_sb = vp.tile([P, SB, E], BF, tag="v")
            for tb in range(SB):
                for nb in range(2):
                    ps_v = psB.tile([P, HALF], FP, tag="big")
                    for ko in range(KO):
                        nc.tensor.matmul(ps_v[:],
                                         lhsT=hn[:, ko, tb * P:(tb + 1) * P],
                                         rhs=wv_sb[:, ko, nb * HALF:(nb + 1) * HALF],
                                         start=(ko == 0), stop=(ko == KO - 1))
                    nc.any.tensor_copy(v_sb[:, tb, nb * HALF:(nb + 1) * HALF],
                                       ps_v[:])

            # z per half: partial sums over tb, fp32r ones-matmul for the
            # cross-partition reduction -> z replicated [P, s-half].
            zsbs = []
            for sh in range(2):
                zs = stp.tile([P, HALF], FR, tag="zs")
                nc.gpsimd.tensor_tensor(zs[:], wT[:, 0, sh * HALF:(sh + 1) * HALF],
                                        wT[:, 1, sh * HALF:(sh + 1) * HALF], OP.add)
                for tb in range(2, SB):
                    nc.gpsimd.tensor_tensor(zs[:], zs[:],
                                            wT[:, tb, sh * HALF:(sh + 1) * HALF],
                                            OP.add)
                ps_z = psZ.tile([P, HALF], FP, tag="z")
                nc.tensor.matmul(ps_z[:], lhsT=onesR[:],
                                 rhs=zs[:], start=True, stop=True)
                z_sb = otp.tile([P, HALF], FP, tag="zsb")
                nc.vector.tensor_copy(z_sb[:], ps_z[:])
                zsbs.append(z_sb)

            if final:
                # transpose z into per-row columns; invz = 1/z
                zc = stp.tile([P, SB], FP, tag="zc")
                for sh in range(2):
                    ps_t = psZ.tile([P, HALF], FP, tag="z")
                    for sbb in range(4):
                        nc.tensor.transpose(ps_t[:, sbb * P:(sbb + 1) * P],
                                            zsbs[sh][:, sbb * P:(sbb + 1) * P],
                                            identF[:])
                        nc.vector.tensor_copy(zc[:, sh * 4 + sbb:sh * 4 + sbb + 1],
                                              ps_t[:, sbb * P:sbb * P + 1])
                invzc = stp.tile([P, SB], FP, tag="invzc")
                nc.vector.reciprocal(invzc[:], zc[:])
                # out: [s_block, o] = sum_tb wT_blk^T @ v ; * 1/z ; -> DRAM
                for blk in range(SB):
                    for nb in range(2):
                        ps_o = psB.tile([P, HALF], FP, tag="big")
                        for tb in range(SB):
                            nc.tensor.matmul(ps_o[:],
                                             lhsT=wT[:, tb, blk * P:(blk + 1) * P],
                                             rhs=v_sb[:, tb, nb * HALF:(nb + 1) * HALF],
                                             start=(tb == 0), stop=(tb == SB - 1))
                        ot = otp.tile([P, HALF], FP, tag="ot")
                        nc.vector.tensor_scalar_mul(ot[:], ps_o[:],
                                                    invzc[:, blk:blk + 1])
                        nc.sync.dma_start(
                            out_d.ap()[head, blk * P:(blk + 1) * P,
                                       nb * HALF:(nb + 1) * HALF],
                            ot[:])
                return None

            # not final: out in [E,S] layout (lhsT = v slice), then the fused
            # LayerNorm with replicated stats -> hn_next [P, KO, S] bf16.
            raw = htp.tile([P, KO, S], FP, tag="raw")
            hn_next = hn1p.tile([P, KO, S], BF, tag="hn1")
            for sh in range(2):
                ssl = slice(sh * HALF, (sh + 1) * HALF)
                for ob in range(KO):
                    ps_o = psB.tile([P, HALF], FP, tag="big")
                    for tb in range(SB):
                        nc.tensor.matmul(ps_o[:],
                                         lhsT=v_sb[:, tb, ob * P:(ob + 1) * P],
                                         rhs=wT[:, tb, ssl],
                                         start=(tb == 0), stop=(tb == SB - 1))
                    nc.any.tensor_copy(raw[:, ob, ssl], ps_o[:])
                # mean: (1/E) * ones-matmul over partial sums (fp32r)
                musum = lnp.tile([P, HALF], FR, tag="musum")
                nc.vector.tensor_tensor(musum[:], raw[:, 0, ssl], raw[:, 1, ssl],
                                        OP.add)
                for ob in range(2, KO):
                    nc.vector.tensor_tensor(musum[:], musum[:], raw[:, ob, ssl],
                                            OP.add)
                # sum of squares (gpsimd)
                sqsum = lnp.tile([P, HALF], FR, tag="sqsum")
                sqt = lnp.tile([P, HALF], FP, tag="sqt")
                nc.gpsimd.tensor_tensor(sqsum[:], raw[:, 0, ssl], raw[:, 0, ssl],
                                        OP.mult)
                for ob in range(1, KO):
                    nc.gpsimd.tensor_tensor(sqt[:], raw[:, ob, ssl],
                                            raw[:, ob, ssl], OP.mult)
                    nc.gpsimd.tensor_tensor(sqsum[:], sqsum[:], sqt[:], OP.add)
                ps_mu = psZ.tile([P, HALF], FP, tag="z")
                nc.tensor.matmul(ps_mu[:], lhsT=oneER[:], rhs=musum[:],
                                 start=True, stop=True)
                mu = lnp.tile([P, HALF], FP, tag="mu")
                nc.vector.tensor_copy(mu[:], ps_mu[:])
                ps_sq = psZ.tile([P, HALF], FP, tag="z")
                nc.tensor.matmul(ps_sq[:], lhsT=oneER[:], rhs=sqsum[:],
                                 start=True, stop=True)
                # var_raw + eps*z^2 -> rstd
                var = lnp.tile([P, HALF], FP, tag="var")
                nc.vector.tensor_tensor(var[:], mu[:], mu[:], OP.mult)
                nc.vector.tensor_tensor(var[:], ps_sq[:], var[:], OP.subtract)
                zq = lnp.tile([P, HALF], FP, tag="zq")
                nc.gpsimd.tensor_tensor(zq[:], zsbs[sh][:], zsbs[sh][:], OP.mult)
                nc.gpsimd.tensor_scalar(zq[:], zq[:], scalar1=float(EPS),
                                        scalar2=None, op0=OP.mult)
                nc.vector.tensor_tensor(var[:], var[:], zq[:], OP.add)
                sd = lnp.tile([P, HALF], FP, tag="sdr")
                nc.scalar.activation(sd[:], var[:], AF.Sqrt)
                nc.vector.reciprocal(var[:], sd[:])
                rstd = var  # reciprocal written in place over the var tile
                # normalize: hn_next = (raw - mu) * rstd (bf16 intermediate)
                for ob in range(KO):
                    eng = nc.vector if ob % 2 == 0 else nc.gpsimd
                    eng.tensor_tensor(hn_next[:, ob, ssl], raw[:, ob, ssl],
                                      mu[:], OP.subtract)
                    eng.tensor_tensor(hn_next[:, ob, ssl], hn_next[:, ob, ssl],
                                      rstd[:], OP.mult)
                    if not g1_identity:
                        nc.vector.tensor_scalar(hn_next[:, ob, ssl],
                                                hn_next[:, ob, ssl],
                                                scalar1=g1c_t[:, ob:ob + 1],
                                                scalar2=b1c_t[:, ob:ob + 1],
                                                op0=OP.mult, op1=OP.add)
            return hn_next

        # ---------------- per-head: layer1 -> (fused LN) -> layer2 ----------
        for head in range(NH):
            hn1T = attn_unit(0, head, hn0T, final=False)
            attn_unit(1, head, hn1T, final=True)

    if legalize:
        _legalize_multi_waits(nc)
    return nc


_CACHE = {}


def _get_nc(g0_identity, g1_identity, legalize=True):
    key = (g0_identity, g1_identity, legalize)
    if key not in _CACHE:
        _CACHE[key] = _build_nc(g0_identity, g1_identity, legalize)
    return _CACHE[key]


def _prep_in_maps(x, emb, ln_gamma, ln_beta, Wq, Wk, Wv):
    x = np.asarray(x)
    bf = ml_dtypes.bfloat16
    emb = np.ascontiguousarray(np.asarray(emb, dtype=np.float32).astype(bf))
    ln_gamma = np.asarray(ln_gamma, dtype=np.float32)
    ln_beta = np.asarray(ln_beta, dtype=np.float32)
    Wq = np.asarray(Wq, dtype=np.float32)
    Wk = np.asarray(Wk, dtype=np.float32)
    Wv = np.asarray(Wv, dtype=np.float32)

    # [L,H,E,2A] packed (WqT | WkT); [L,H,E,E] = WvT -- bf16
    wqkT = np.concatenate([Wq.transpose(0, 1, 3, 2), Wk.transpose(0, 1, 3, 2)],
                          axis=3).astype(bf)
    wvT = Wv.transpose(0, 1, 3, 2).astype(bf)

    in_maps = []
    for c in range(8):
        b = c // 2
        hs = (c % 2) * NH
        in_maps.append({
            "emb": emb,
            "xidx": np.ascontiguousarray(x[b].astype(np.int32).reshape(S, 1)),
            "wqk": np.ascontiguousarray(wqkT[:, hs:hs + NH]),
            "wv": np.ascontiguousarray(wvT[:, hs:hs + NH]),
            "g0": np.ascontiguousarray(ln_gamma[0]),
            "b0": np.ascontiguousarray(ln_beta[0]),
            "g1": np.ascontiguousarray(ln_gamma[1]),
            "b1": np.ascontiguousarray(ln_beta[1]),
        })
    g0_id = bool(np.all(ln_gamma[0] == 1.0) and np.all(ln_beta[0] == 0.0))
    g1_id = bool(np.all(ln_gamma[1] == 1.0) and np.all(ln_beta[1] == 0.0))
    return in_maps, g0_id, g1_id


def run(inputs, trace=False, trace_cores=None):
    in_maps, g0_id, g1_id = _prep_in_maps(**inputs)
    nc = _get_nc(g0_id, g1_id)
    res = run_bass_kernel_spmd(nc, in_maps, core_ids=list(range(8)),
                               trace=trace, trace_cores=trace_cores)
    out = np.empty((B, H, S, E), dtype=np.float32)
    for c in range(8):
        out[c // 2, (c % 2) * NH:(c % 2) * NH + NH] = res.results[c]["out"]
    return out, res


def kernel(x, emb, ln_gamma, ln_beta, Wq, Wk, Wv):
    out, _ = run(dict(x=x, emb=emb, ln_gamma=ln_gamma, ln_beta=ln_beta,
                      Wq=Wq, Wk=Wk, Wv=Wv))
    return out


# revision 13
# speedup vs baseline: 1.0880x; 1.0880x over previous
"""Trainium2 Bass kernel for nn_EruSelfAttentionModel (B=4,S=1024,E=1024,A=64,H=16,L=2).

Sharding: 8 cores; core c handles batch c//2 and heads (c%2)*8..(c%2)*8+8.
Heads are independent through both layers, so each core runs its (batch,
8-head) slice end-to-end with no collectives.

v2 (this file): all matmuls in bf16 (4x PE throughput vs fp32), both layers
fused per head (no DRAM round-trip for the inter-layer activations), and the
layer-1 softmax division is folded into the inter-layer LayerNorm via LN's
scale invariance: LN(out/z) == LN(out_raw) with the eps bias corrected to
eps*z^2 per row (z columns obtained by PE-transposing the ones-matmul z).

Per-core dataflow per head:
  stage0 (once): indirect-DMA gather of bf16 embeddings -> LN (bn_stats) ->
      PE transpose -> hn0T [E,S] bf16 in SBUF (shared by all 8 heads).
  layer 1: qkT / scores+exp(wT bf16) / v(bf16) / z (fp32r ones-matmul) /
      out_raw [S,E] fp32 in SBUF -> LN (eps*z^2 bias) -> PE transpose ->
      hn1T [E,S] bf16.
  layer 2: same attention, final out = psum * (1/z) column, fp32 -> DRAM.
"""

import math
import os
from contextlib import ExitStack

import numpy as np
import ml_dtypes

# The device path (bass2jax under axon) needs the axon PJRT backend; a
# JAX_PLATFORMS=cpu pin (common for running the jax reference) would break it.
if "JAX_PLATFORMS" in os.environ and "axon" not in os.environ["JAX_PLATFORMS"]:
    del os.environ["JAX_PLATFORMS"]

import bass_rust
from bass_rust import SyncInfo
import concourse.bass as bass
import concourse.mybir as mybir
import concourse.tile as tile
from concourse.bass_utils import run_bass_kernel_spmd
from concourse.masks import make_identity

B, S, E, A, H, L, V = 4, 1024, 1024, 64, 16, 2, 32000
EPS = 1e-5
SCALE = math.sqrt(E)
P = 128
KO = E // P       # 8 k-blocks over E
SB = S // P       # 8 s-blocks
NH = H // 2       # 8 heads per core
HALF = S // 2     # 512
TA = 2 * A        # 128 (packed q|k)
FP = mybir.dt.float32
FR = mybir.dt.float32r
BF = mybir.dt.bfloat16
AF = mybir.ActivationFunctionType
OP = mybir.AluOpType

_WID = [0]


def _legalize_multi_waits(nc, max_keep=1):
    """This walrus build accepts at most one sync-wait command per engine
    instruction; split extras into standalone EventSemaphore waits."""
    for f in nc.m.functions:
        for blk in f.blocks:
            out = []
            changed = False
            for inst in blk.instructions:
                si = inst.sync_info
                ow = list(si.on_wait) if si is not None else []
                if len(ow) > max_keep:
                    changed = True
                    for w in ow[:-max_keep]:
                        _WID[0] += 1
                        out.append(bass_rust.InstEventSemaphore(
                            name=f"WSPLIT-{_WID[0]}",
                            engine=inst.engine,
                            ins=[], outs=[],
                            sync_info=SyncInfo(on_wait=[w], on_update=[]),
                        ))
                    inst.sync_info = SyncInfo(on_wait=ow[-max_keep:],
                                              on_update=list(si.on_update))
                out.append(inst)
            if changed:
                blk.instructions = out


def _build_nc(g0_identity, g1_identity, legalize=True):
    nc = bass.Bass("TRN2")

    emb = nc.dram_tensor("emb", [V, E], BF, kind="ExternalInput")
    xidx = nc.dram_tensor("xidx", [S, 1], mybir.dt.int32, kind="ExternalInput")
    wqk = nc.dram_tensor("wqk", [L, NH, E, TA], BF, kind="ExternalInput")
    wv = nc.dram_tensor("wv", [L, NH, E, E], BF, kind="ExternalInput")
    g0 = nc.dram_tensor("g0", [E], FP, kind="ExternalInput")
    b0 = nc.dram_tensor("b0", [E], FP, kind="ExternalInput")
    g1 = nc.dram_tensor("g1", [E], FP, kind="ExternalInput")
    b1 = nc.dram_tensor("b1", [E], FP, kind="ExternalInput")
    out_d = nc.dram_tensor("out", [NH, S, E], FP, kind="ExternalOutput")

    with tile.TileContext(nc) as tc, ExitStack() as ctx:
        const = ctx.enter_context(tc.tile_pool(name="const", bufs=1))
        hn0p = ctx.enter_context(tc.tile_pool(name="hn0p", bufs=1))
        wqkp = ctx.enter_context(tc.tile_pool(name="wqkp", bufs=2))
        wvp = ctx.enter_context(tc.tile_pool(name="wvp", bufs=2))
        vp = ctx.enter_context(tc.tile_pool(name="vp", bufs=2))
        wtp = ctx.enter_context(tc.tile_pool(name="wtp", bufs=2))
        qkp = ctx.enter_context(tc.tile_pool(name="qkp", bufs=2))
        htp = ctx.enter_context(tc.tile_pool(name="htp", bufs=2))
        hnsp = ctx.enter_context(tc.tile_pool(name="hnsp", bufs=1))
        stp = ctx.enter_context(tc.tile_pool(name="stp", bufs=2))
        otp = ctx.enter_context(tc.tile_pool(name="otp", bufs=2))
        lnp = ctx.enter_context(tc.tile_pool(name="lnp", bufs=1))
        lnp2 = ctx.enter_context(tc.tile_pool(name="lnp2", bufs=2))
        psB = ctx.enter_context(tc.tile_pool(name="psB", bufs=3, space="PSUM"))
        psS = ctx.enter_context(tc.tile_pool(name="psS", bufs=2, space="PSUM"))
        psZ = ctx.enter_context(tc.tile_pool(name="psZ", bufs=2, space="PSUM"))
        psT = ctx.enter_context(tc.tile_pool(name="psT", bufs=1, space="PSUM"))

        identB = const.tile([P, P], BF)
        make_identity(nc, identB[:])
        identF = const.tile([P, P], FP)
        make_identity(nc, identF[:])
        onesR = const.tile([P, P], FR)
        onesF = const.tile([P, P], FP)
        nc.vector.memset(onesF[:], 1.0)
        nc.vector.tensor_copy(onesR[:], onesF[:])
        oneER = const.tile([P, P], FR)
        nc.vector.memset(onesF[:], 1.0 / E)
        nc.vector.tensor_copy(oneER[:], onesF[:])
        eps_t = const.tile([P, 1], FP)
        nc.vector.memset(eps_t[:], EPS)
        if not g0_identity:
            g0rep = const.tile([P, E], FP)
            b0rep = const.tile([P, E], FP)
            nc.sync.dma_start(g0rep[:], g0.ap()[None, :].to_broadcast([P, E]))
            nc.sync.dma_start(b0rep[:], b0.ap()[None, :].to_broadcast([P, E]))
        if not g1_identity:
            g1c_t = const.tile([P, KO], FP)
            b1c_t = const.tile([P, KO], FP)
            nc.sync.dma_start(g1c_t[:], g1.ap().rearrange("(ko p) -> p ko", p=P))
            nc.sync.dma_start(b1c_t[:], b1.ap().rearrange("(ko p) -> p ko", p=P))

        # ---------------- stage 0: embed + LN0 + transpose -> hn0T ----------
        hn0T = hn0p.tile([P, KO, S], BF, tag="hn0")  # [e_inner, e_outer, s]
        for sb in range(SB):
            idxt = stp.tile([P, 1], mybir.dt.int32, tag="idx")
            nc.sync.dma_start(idxt[:], xidx[sb * P:(sb + 1) * P, :])
            h0sb = hnsp.tile([P, E], BF, tag="h0")
            nc.gpsimd.indirect_dma_start(
                out=h0sb[:], out_offset=None, in_=emb[:, :],
                in_offset=bass.IndirectOffsetOnAxis(ap=idxt[:, :1], axis=0),
            )
            stats = stp.tile([P, 2, 6], FP, tag="bnst")
            nc.vector.bn_stats(stats[:, 0, :], h0sb[:, 0:HALF])
            nc.vector.bn_stats(stats[:, 1, :], h0sb[:, HALF:S])
            mv = stp.tile([P, 2], FP, tag="bnmv")
            nc.vector.bn_aggr(mv[:], stats[:])
            sd = stp.tile([P, 1], FP, tag="sd")
            nc.scalar.activation(sd[:], mv[:, 1:2], AF.Sqrt, bias=eps_t[:])
            rstd = stp.tile([P, 1], FP, tag="rstd")
            nc.vector.reciprocal(rstd[:], sd[:])
            nc.vector.tensor_scalar(h0sb[:], h0sb[:], scalar1=mv[:, 0:1],
                                    scalar2=rstd[:], op0=OP.subtract,
                                    op1=OP.mult)
            if not g0_identity:
                nc.vector.tensor_tensor(h0sb[:], h0sb[:], g0rep[:], OP.mult)
                nc.vector.tensor_tensor(h0sb[:], h0sb[:], b0rep[:], OP.add)
            for eo in range(0, KO, 4):
                pst = psT.tile([P, 4, P], BF, tag="pst")
                for j in range(4):
                    nc.tensor.transpose(pst[:, j, :],
                                        h0sb[:, (eo + j) * P:(eo + j + 1) * P],
                                        identB[:])
                nc.any.tensor_copy(hn0T[:, eo:eo + 4, sb * P:(sb + 1) * P],
                                   pst[:, :, :])

        # ---------------- attention unit ------------------------------------
        # hn: [P, KO, S] bf16 in [E,S] layout. When rstd_info is given, hn is
        # only mean-centered and the missing *rstd(row) factor is folded in:
        # q gets *rstd(s) (2 DVE ops), the exp pre-scale column provides
        # rstd(t)/SCALE, and the v PSUM->SBUF copy multiplies by rstd(t).
        # (The attention output is invariant to a *global* rescale of
        # exp-weights only via z, and all rstd factors are applied exactly.)
        #
        # final=False: produces the next layer's mean-centered hn (in place
        # over its raw tile), the per-sh rstd tiles, and the rstd/SCALE
        # columns; final=True: scales by 1/z and DMAs [S,E] fp32 to out_d.
        def attn_unit(layer, head, hn, rstd_info, final):
            wqk_sb = wqkp.tile([P, KO, TA], BF, tag="wqk")
            nc.sync.dma_start(wqk_sb[:],
                              wqk.ap()[layer, head].rearrange("(ko p) m -> p ko m", p=P))
            wv_sb = wvp.tile([P, KO, E], BF, tag="wv")
            nc.sync.dma_start(wv_sb[:],
                              wv.ap()[layer, head].rearrange("(ko p) o -> p ko o", p=P))
            if rstd_info is not None:
                rstd_t, rstdc32, rstdc = rstd_info

            # qkT: q rows 0:64 of qT (junk rows 64:128, finite), k rows 0:64
            # of kT (rows 64:128 zeroed) -> K=128 scores matmul keeps FWL.
            qT = qkp.tile([P, S], BF, tag="qT")
            kT = qkp.tile([P, S], BF, tag="kT")
            nc.gpsimd.memset(kT[A:P, :], 0.0)
            for nb in range(2):
                ps_qk = psB.tile([P, HALF], FP, tag="big")
                for ko in range(KO):
                    nc.tensor.matmul(ps_qk[:],
                                     lhsT=wqk_sb[:, ko, :],
                                     rhs=hn[:, ko, nb * HALF:(nb + 1) * HALF],
                                     start=(ko == 0), stop=(ko == KO - 1))
                if rstd_info is not None:
                    nc.vector.tensor_tensor(ps_qk[0:A, :], ps_qk[0:A, :],
                                            rstd_t[0:A, nb, :], OP.mult)
                nc.scalar.copy(qT[:, nb * HALF:(nb + 1) * HALF], ps_qk[:, :])
                nc.scalar.copy(kT[0:A, nb * HALF:(nb + 1) * HALF], ps_qk[A:TA, :])

            # scoresT + exp -> wT [t_inner, tb, s] bf16
            wT = wtp.tile([P, SB, S], BF, tag="wT")
            for sh in range(2):
                for tb in range(SB):
                    ps_s = psS.tile([P, HALF], FP, tag="s")
                    nc.tensor.matmul(ps_s[:], lhsT=kT[:, tb * P:(tb + 1) * P],
                                     rhs=qT[:, sh * HALF:(sh + 1) * HALF],
                                     start=True, stop=True)
                    if rstd_info is not None:
                        nc.scalar.activation(wT[:, tb, sh * HALF:(sh + 1) * HALF],
                                             ps_s[:], AF.Exp,
                                             scale=rstdc32[:, tb:tb + 1])
                    else:
                        nc.scalar.activation(wT[:, tb, sh * HALF:(sh + 1) * HALF],
                                             ps_s[:], AF.Exp,
                                             scale=float(1.0 / SCALE))

            # v: [t_inner, tb, o] bf16 (*rstd(t) when deferred)
            v_sb = vp.tile([P, SB, E], BF, tag="v")
            for tb in range(SB):
                for nb in range(2):
                    ps_v = psB.tile([P, HALF], FP, tag="big")
                    for ko in range(KO):
                        nc.tensor.matmul(ps_v[:],
                                         lhsT=hn[:, ko, tb * P:(tb + 1) * P],
                                         rhs=wv_sb[:, ko, nb * HALF:(nb + 1) * HALF],
                                         start=(ko == 0), stop=(ko == KO - 1))
                    if rstd_info is not None:
                        nc.any.tensor_scalar(
                            v_sb[:, tb, nb * HALF:(nb + 1) * HALF], ps_v[:],
                            scalar1=rstdc[:, tb:tb + 1], scalar2=None,
                            op0=OP.mult)
                    else:
                        nc.any.tensor_copy(v_sb[:, tb, nb * HALF:(nb + 1) * HALF],
                                           ps_v[:])

            # z per half: partial sums over tb + fp32r ones-matmul -> z
            # replicated [P, s-half]
            zsbs = []
            for sh in range(2):
                zs = stp.tile([P, HALF], FR, tag="zs")
                nc.gpsimd.tensor_tensor(zs[:], wT[:, 0, sh * HALF:(sh + 1) * HALF],
                                        wT[:, 1, sh * HALF:(sh + 1) * HALF], OP.add)
                for tb in range(2, SB):
                    nc.gpsimd.tensor_tensor(zs[:], zs[:],
                                            wT[:, tb, sh * HALF:(sh + 1) * HALF],
                                            OP.add)
                ps_z = psZ.tile([P, HALF], FP, tag="z")
                nc.tensor.matmul(ps_z[:], lhsT=onesR[:],
                                 rhs=zs[:], start=True, stop=True)
                z_sb = otp.tile([P, HALF], FP, tag="zsb")
                nc.vector.tensor_copy(z_sb[:], ps_z[:])
                zsbs.append(z_sb)

            if final:
                # transpose z into per-row columns; invz = 1/z
                zc = stp.tile([P, SB], FP, tag="zc")
                for sh in range(2):
                    ps_t = psZ.tile([P, HALF], FP, tag="z")
                    for sbb in range(4):
                        nc.tensor.transpose(ps_t[:, sbb * P:(sbb + 1) * P],
                                            zsbs[sh][:, sbb * P:(sbb + 1) * P],
                                            identF[:])
                        nc.vector.tensor_copy(zc[:, sh * 4 + sbb:sh * 4 + sbb + 1],
                                              ps_t[:, sbb * P:sbb * P + 1])
                invzc = stp.tile([P, SB], FP, tag="invzc")
                nc.vector.reciprocal(invzc[:], zc[:])
                # out: [s_block, o] = sum_tb wT_blk^T @ v ; * 1/z ; -> DRAM
                for blk in range(SB):
                    for nb in range(2):
                        ps_o = psB.tile([P, HALF], FP, tag="big")
                        for tb in range(SB):
                            nc.tensor.matmul(ps_o[:],
                                             lhsT=wT[:, tb, blk * P:(blk + 1) * P],
                                             rhs=v_sb[:, tb, nb * HALF:(nb + 1) * HALF],
                                             start=(tb == 0), stop=(tb == SB - 1))
                        ot = otp.tile([P, HALF], FP, tag="ot")
                        nc.vector.tensor_scalar_mul(ot[:], ps_o[:],
                                                    invzc[:, blk:blk + 1])
                        nc.sync.dma_start(
                            out_d.ap()[head, blk * P:(blk + 1) * P,
                                       nb * HALF:(nb + 1) * HALF],
                            ot[:])
                return None

            # not final: out in [E,S] layout, then mean-center in place and
            # derive rstd (z-free LN: rstd = 1/sqrt(var_raw + eps*z^2)).
            raw = htp.tile([P, KO, S], BF, tag="raw")
            rstd_t = lnp2.tile([P, 2, HALF], FP, tag="rstd_t")
            rstdc32n = lnp2.tile([P, SB], FP, tag="rstdc32")
            rstdcn = lnp2.tile([P, SB], FP, tag="rstdc")
            mu_t = lnp2.tile([P, 2, HALF], FP, tag="mu_t")
            for sh in range(2):
                ssl = slice(sh * HALF, (sh + 1) * HALF)
                for ob in range(KO):
                    ps_o = psB.tile([P, HALF], FP, tag="big")
                    for tb in range(SB):
                        nc.tensor.matmul(ps_o[:],
                                         lhsT=v_sb[:, tb, ob * P:(ob + 1) * P],
                                         rhs=wT[:, tb, ssl],
                                         start=(tb == 0), stop=(tb == SB - 1))
                    nc.any.tensor_copy(raw[:, ob, ssl], ps_o[:])
                # mean and sum-of-squares via fp32r ones-matmuls
                musum = lnp.tile([P, HALF], FR, tag="musum")
                nc.vector.tensor_tensor(musum[:], raw[:, 0, ssl], raw[:, 1, ssl],
                                        OP.add)
                for ob in range(2, KO):
                    nc.vector.tensor_tensor(musum[:], musum[:], raw[:, ob, ssl],
                                            OP.add)
                sqsum = lnp.tile([P, HALF], FR, tag="sqsum")
                sqt = lnp.tile([P, HALF], FP, tag="sqt")
                nc.gpsimd.tensor_tensor(sqsum[:], raw[:, 0, ssl], raw[:, 0, ssl],
                                        OP.mult)
                for ob in range(1, KO):
                    nc.gpsimd.tensor_tensor(sqt[:], raw[:, ob, ssl],
                                            raw[:, ob, ssl], OP.mult)
                    nc.gpsimd.tensor_tensor(sqsum[:], sqsum[:], sqt[:], OP.add)
                ps_mu = psZ.tile([P, HALF], FP, tag="z")
                nc.tensor.matmul(ps_mu[:], lhsT=oneER[:], rhs=musum[:],
                                 start=True, stop=True)
                nc.vector.tensor_copy(mu_t[:, sh, :], ps_mu[:])
                ps_sq = psZ.tile([P, HALF], FP, tag="z")
                nc.tensor.matmul(ps_sq[:], lhsT=oneER[:], rhs=sqsum[:],
                                 start=True, stop=True)
                # var_raw + eps*z^2 -> rstd
                var = lnp.tile([P, HALF], FP, tag="var")
                nc.vector.tensor_tensor(var[:], mu_t[:, sh, :], mu_t[:, sh, :],
                                        OP.mult)
                nc.vector.tensor_tensor(var[:], ps_sq[:], var[:], OP.subtract)
                zq = lnp.tile([P, HALF], FP, tag="zq")
                nc.gpsimd.tensor_tensor(zq[:], zsbs[sh][:], zsbs[sh][:], OP.mult)
                nc.gpsimd.tensor_scalar(zq[:], zq[:], scalar1=float(EPS),
                                        scalar2=None, op0=OP.mult)
                nc.vector.tensor_tensor(var[:], var[:], zq[:], OP.add)
                sd = lnp.tile([P, HALF], FP, tag="sdr")
                nc.scalar.activation(sd[:], var[:], AF.Sqrt)
                nc.vector.reciprocal(rstd_t[:, sh, :], sd[:])
                # mean-center raw in place (the *rstd factor is deferred)
                for ob in range(KO):
                    eng = nc.vector if ob % 2 == 0 else nc.gpsimd
                    eng.tensor_tensor(raw[:, ob, ssl], raw[:, ob, ssl],
                                      mu_t[:, sh, :], OP.subtract)
                # rstd columns: [t,1] per t-block via PE transpose
                ps_t = psZ.tile([P, HALF], FP, tag="z")
                for sbb in range(4):
                    nc.tensor.transpose(ps_t[:, sbb * P:(sbb + 1) * P],
                                        rstd_t[:, sh, sbb * P:(sbb + 1) * P],
                                        identF[:])
                    nc.vector.tensor_copy(
                        rstdcn[:, sh * 4 + sbb:sh * 4 + sbb + 1],
                        ps_t[:, sbb * P:sbb * P + 1])
            nc.vector.tensor_scalar(rstdc32n[:], rstdcn[:],
                                    scalar1=float(1.0 / SCALE), scalar2=None,
                                    op0=OP.mult)
            return raw, (rstd_t, rstdc32n, rstdcn)

        # ------------- per-head, software-pipelined across heads ------------
        # l1(h) ... l1(h+1) ... l2(h): head h+1's layer-1 PE work hides the
        # engine-side LN-stats chain of head h.
        pending = None
        for head in range(NH):
            raw, rinfo = attn_unit(0, head, hn0T, None, final=False)
            if pending is not None:
                attn_unit(1, pending[0], pending[1], pending[2], final=True)
            pending = (head, raw, rinfo)
        attn_unit(1, pending[0], pending[1], pending[2], final=True)

    if legalize:
        _legalize_multi_waits(nc)
    return nc


_CACHE = {}


def _get_nc(g0_identity, g1_identity, legalize=True):
    key = (g0_identity, g1_identity, legalize)
    if key not in _CACHE:
        _CACHE[key] = _build_nc(g0_identity, g1_identity, legalize)
    return _CACHE[key]


def _prep_in_maps(x, emb, ln_gamma, ln_beta, Wq, Wk, Wv):
    x = np.asarray(x)
    bf = ml_dtypes.bfloat16
    emb = np.ascontiguousarray(np.asarray(emb, dtype=np.float32).astype(bf))
    ln_gamma = np.asarray(ln_gamma, dtype=np.float32)
    ln_beta = np.asarray(ln_beta, dtype=np.float32)
    Wq = np.asarray(Wq, dtype=np.float32)
    Wk = np.asarray(Wk, dtype=np.float32)
    Wv = np.asarray(Wv, dtype=np.float32)

    # [L,H,E,2A] packed (WqT | WkT); [L,H,E,E] = WvT -- bf16
    wqkT = np.concatenate([Wq.transpose(0, 1, 3, 2), Wk.transpose(0, 1, 3, 2)],
                          axis=3).astype(bf)
    wvT = Wv.transpose(0, 1, 3, 2).astype(bf)

    in_maps = []
    for c in range(8):
        b = c // 2
        hs = (c % 2) * NH
        in_maps.append({
            "emb": emb,
            "xidx": np.ascontiguousarray(x[b].astype(np.int32).reshape(S, 1)),
            "wqk": np.ascontiguousarray(wqkT[:, hs:hs + NH]),
            "wv": np.ascontiguousarray(wvT[:, hs:hs + NH]),
            "g0": np.ascontiguousarray(ln_gamma[0]),
            "b0": np.ascontiguousarray(ln_beta[0]),
            "g1": np.ascontiguousarray(ln_gamma[1]),
            "b1": np.ascontiguousarray(ln_beta[1]),
        })
    g0_id = bool(np.all(ln_gamma[0] == 1.0) and np.all(ln_beta[0] == 0.0))
    g1_id = bool(np.all(ln_gamma[1] == 1.0) and np.all(ln_beta[1] == 0.0))
    return in_maps, g0_id, g1_id


def run(inputs, trace=False, trace_cores=None):
    in_maps, g0_id, g1_id = _prep_in_maps(**inputs)
    nc = _get_nc(g0_id, g1_id)
    res = run_bass_kernel_spmd(nc, in_maps, core_ids=list(range(8)),
                               trace=trace, trace_cores=trace_cores)
    out = np.empty((B, H, S, E), dtype=np.float32)
    for c in range(8):
        out[c // 2, (c % 2) * NH:(c % 2) * NH + NH] = res.results[c]["out"]
    return out, res


def kernel(x, emb, ln_gamma, ln_beta, Wq, Wk, Wv):
    out, _ = run(dict(x=x, emb=emb, ln_gamma=ln_gamma, ln_beta=ln_beta,
                      Wq=Wq, Wk=Wk, Wv=Wv))
    return out
